# revision 17
# baseline (speedup 1.0000x reference)
"""Multi-head attention (B=4, S=2048, D=512, H=8) on 8 Trainium2 NeuronCores.

Sharding: core c handles batch b = c//2 and heads [4*(c%2) .. 4*(c%2)+3]
(data parallel on B, tensor parallel on H). Each core computes Q/K/V
projections for its 4 heads, per-head attention, and a partial output
projection (its 256 rows of Wo). The host sums the two partial outputs per
batch and adds bo.

Perf design (v4, all-bf16):
 - All matmul operands bf16 (fp32 lowers to two half-speed PE passes = 4
   cyc/col; bf16 streams at 1). PSUM accumulation stays fp32. fp8 was
   tried and rejected: quantization noise does not average out for queries
   with concentrated softmax (rel err 5e-2 > the 2e-2 gate).
 - Key compaction on host: masked keys contribute nothing (their V' rows
   incl. the ones-column are zero), so only unmasked keys ship for the K/V
   side, padded to a multiple of 128. ~2x less scores/exp/AV work.
 - The compacted x^T carries an indicator row (1=real key) and wv' a
   matching entry, so V's mask/ones column falls out of the projection -
   no device-side mask work at all.
 - exp runs on the Scalar(ACT) engine over [128,3,512] PSUM spans (three
   key chunks per instruction) to amortize the ~190ns/instr access
   latency. Scores are computed transposed (keys on partitions); softmax
   max-subtraction is skipped (logits ~N(0,1), fp32 psum cannot overflow).
 - The in-order PE stalls whenever it waits on ACT; a filler queue emits
   projection / output-projection chains at those points, and each head's
   last AV group + copy-out ride inside the NEXT head's first group
   (cross-head software pipeline), so the PE never idles. An idle PE also
   drops out of its 2.4GHz p-state, which doubles matmul time.
 - All DMAs are issued from the otherwise-idle GpSimd queue (~25ns issue
   vs ~565ns on the Sync engine, whose serial issue dominated the ramp).
 - Normalization is DRAM-free: the denominator row (V'-ones column of the
   AV matmul) is reciprocal'd on DVE into a single partition, broadcast
   across 64 hd partitions by a K=1 outer-product matmul into PSUM, and
   multiplied into the O tiles; the output projection rides the filler
   queue right behind it.
"""

import numpy as np
import ml_dtypes
from contextlib import ExitStack

import concourse.bass as bass
from concourse.bacc import Bacc
import concourse.mybir as mybir
import concourse.tile as tile
from concourse import bass_utils

F32 = mybir.dt.float32
BF16 = mybir.dt.bfloat16
NPBF16 = ml_dtypes.bfloat16

B, S, D, H, HD = 4, 2048, 512, 8, 64
P = 128
HPC = 4            # heads per core
NS = S // 512      # 4 query blocks of 512
VB = 65            # V' head block: 64 hd cols + the ones/indicator column


def _nblocks(total, step=512):
    return [(o, min(step, total - o)) for o in range(0, total, step)]


def _build(aug: bool, nskc: int) -> bass.Bass:
    kq = 5 if aug else 4           # x^T chunks for the Q/K projections
    sk = nskc * P                  # compacted+padded key count
    # exp groups of <=3 chunks (3 psum banks per group, 2 groups in flight)
    groups = []
    c = 0
    while c < nskc:
        n = min(3, nskc - c)
        groups.append((c, n))
        c += n
    nc = Bacc(trn_type="TRN2")

    xT = nc.dram_tensor("xT", [kq * P, S], BF16, kind="ExternalInput")
    xKT = nc.dram_tensor("xKT", [5 * P, sk], BF16, kind="ExternalInput")
    wq = nc.dram_tensor("wq", [kq * P, HPC * HD], BF16, kind="ExternalInput")
    wk = nc.dram_tensor("wk", [kq * P, HPC * HD], BF16, kind="ExternalInput")
    wv = nc.dram_tensor("wv", [5 * P, HPC * VB], BF16, kind="ExternalInput")
    wo = nc.dram_tensor("wo", [2, P, D], BF16, kind="ExternalInput")
    out = nc.dram_tensor("out", [S, D], F32, kind="ExternalOutput")

    with tile.TileContext(nc) as tc, ExitStack() as ctx:
        sb = ctx.enter_context(tc.tile_pool(name="sb", bufs=1))
        apool = ctx.enter_context(tc.tile_pool(name="sc_ps", bufs=2, space="PSUM"))
        avpool = ctx.enter_context(tc.tile_pool(name="av_ps", bufs=2, space="PSUM"))
        dma = nc.gpsimd.dma_start

        _psn = [0]

        def psum512():
            # [128,512] fp32 psum scratch carved from the big "sc" tag
            _psn[0] += 1
            t = apool.tile([P, 3, 512], F32, tag="sc", name=f"ps{_psn[0]}")
            return t[:, 0, :]

        # ---------- input DMAs (split so compute starts early) ----------
        wkt = sb.tile([P, kq, HPC * HD], BF16)
        for t in range(kq):
            dma(wkt[:, t, :], wk[t * P:(t + 1) * P, :])
        xkt = [sb.tile([P, sk], BF16, tag=f"xk{t}", name=f"xk{t}") for t in range(5)]
        for (o, n) in _nblocks(sk):
            for t in range(5):
                dma(xkt[t][:, o:o + n], xKT[t * P:(t + 1) * P, o:o + n])
        wvt = sb.tile([P, 5, HPC * VB], BF16)
        for t in range(5):
            dma(wvt[:, t, :], wv[t * P:(t + 1) * P, :])
        wqt = sb.tile([P, kq, HPC * HD], BF16)
        for t in range(kq):
            dma(wqt[:, t, :], wq[t * P:(t + 1) * P, :])
        xt = [sb.tile([P, S], BF16, tag=f"xt{t}", name=f"xt{t}") for t in range(kq)]
        for (o, n) in _nblocks(S):
            for t in range(kq):
                dma(xt[t][:, o:o + n], xT[t * P:(t + 1) * P, o:o + n])
        wot = [sb.tile([P, D], BF16, tag=f"wo{m}", name=f"wo{m}") for m in range(2)]
        for m in range(2):
            dma(wot[m][:], wo[m])
        ones_t = sb.tile([P, HD], BF16, tag="ones")
        nc.vector.memset(ones_t[:], 1.0)

        # ---------- projections ----------
        ktile = [sb.tile([P, sk], BF16, tag=f"kT{m}", name=f"kT{m}") for m in range(2)]
        qtile = [sb.tile([P, S], BF16, tag=f"qT{m}", name=f"qT{m}") for m in range(2)]
        vt = sb.tile([P, HPC, nskc, VB], BF16)

        def proj_kq(wt, dst, m, src, o, n):
            ps = psum512()
            for t in range(kq):
                nc.tensor.matmul(
                    ps[:, :n], wt[:, t, m * P:(m + 1) * P], src[t][:, o:o + n],
                    start=(t == 0), stop=(t == kq - 1),
                )
            nc.vector.tensor_copy(dst[m][:, o:o + n], ps[:, :n])

        def proj_v(si):
            ps = psum512()
            for t in range(5):
                nc.tensor.matmul(
                    ps[:, :HPC * VB], xkt[t][:, si * P:(si + 1) * P], wvt[:, t, :],
                    start=(t == 0), stop=(t == 4),
                )
            nc.vector.tensor_copy(
                vt[:, :, si, :],
                ps[:, :HPC * VB].rearrange("p (l e) -> p l e", e=VB))

        # ---------- attention ----------
        opair = [sb.tile([P, S], BF16, tag=f"op{m}", name=f"op{m}") for m in range(2)]
        pending = [None]               # previous head's tail (last AVs, copies)
        rrs = {}                       # head parity -> reciprocal tile

        def normalize(m, j):
            # 1/den broadcast to [128, 512] by two K=1 outer products (even
            # head -> partitions 0..63, odd -> 64..127), then one multiply.
            rb = psum512()
            nc.tensor.matmul(rb[0:HD, :], ones_t[HD:HD + 1, :],
                             rrs[0][HD:HD + 1, :], start=True, stop=True)
            nc.tensor.matmul(rb[HD:P, :], ones_t[HD:HD + 1, :],
                             rrs[1][HD:HD + 1, :], start=True, stop=True,
                             skip_group_check=True)
            sl = slice(j * 512, (j + 1) * 512)
            nc.vector.tensor_tensor(opair[m][:, sl], opair[m][:, sl], rb[:],
                                    mybir.AluOpType.mult)

        def attn_head(m, j, h, filler=None, pops=1):
            # Filler units + the previous head's tail are emitted right
            # after the first exp: the in-order PE would otherwise idle
            # there waiting on the ACT engine.
            l = 2 * m + h
            base = h * HD
            qs = qtile[m][base:base + HD, j * 512:(j + 1) * 512]
            av = avpool.tile([VB, 512], F32, tag="av")
            p = sb.tile([P, nskc, 512], BF16, tag="p", bufs=2)
            state = [0]

            def emit_av(upto):
                while state[0] < upto:
                    ck = state[0]
                    nc.tensor.matmul(
                        av[:], vt[:, l, ck, :], p[:, ck, :],
                        start=(ck == 0), stop=(ck == nskc - 1))
                    state[0] += 1

            for gi, (c0, cn) in enumerate(groups):
                sc = apool.tile([P, 3, 512], F32, tag="sc")
                for ci in range(cn):
                    ck = c0 + ci
                    nc.tensor.matmul(
                        sc[:, ci, :],
                        ktile[m][base:base + HD, ck * P:(ck + 1) * P],
                        qs, start=True, stop=True,
                    )
                nc.scalar.activation(p[:, c0:c0 + cn, :], sc[:, :cn, :],
                                     mybir.ActivationFunctionType.Exp,
                                     scale=0.125)
                if filler:
                    for _ in range(pops):
                        if filler:
                            filler.pop(0)()
                if gi == 0 and pending[0] is not None:
                    pending[0]()
                    pending[0] = None
                emit_av(c0)

            def tail():
                emit_av(nskc)
                # O rows into the pair tile (odd head partition-shifted
                # 0..63 -> 64..127 by an SBUF->SBUF DMA) + 1/denominator
                if h == 0:
                    nc.vector.tensor_copy(
                        opair[m][0:HD, j * 512:(j + 1) * 512], av[0:HD, :])
                else:
                    osh = sb.tile([HD, 512], BF16, tag="osh", bufs=2)
                    nc.vector.tensor_copy(osh[:], av[0:HD, :])
                    dma(opair[m][HD:P, j * 512:(j + 1) * 512], osh[:])
                rr = sb.tile([VB, 512], BF16, tag=f"rr{h}", bufs=2,
                             name=f"rr{h}_{m}_{j}")
                with nc.allow_low_precision(reason="per-query 1/den scale; bf16 "
                                            "matches the bf16 O tiles it scales"):
                    nc.vector.reciprocal(rr[HD:VB, :], av[HD:VB, :])
                rrs[h] = rr
                if h == 1:
                    normalize(m, j)

            pending[0] = tail

        def outproj_si(si):
            ps = psum512()
            for mm in range(2):
                nc.tensor.matmul(
                    ps[:], opair[mm][:, si * P:(si + 1) * P], wot[mm][:],
                    start=(mm == 0), stop=(mm == 1),
                )
            osb = sb.tile([P, D], F32, tag="osb", bufs=3)
            nc.vector.tensor_copy(osb[:], ps[:])
            dma(out[si * P:(si + 1) * P, :], osb[:])

        def unit_kq(wt, dst, m, o, n, src):
            return lambda: proj_kq(wt, dst, m, src, o, n)

        # ---------- emission schedule ----------
        # Ramp: first K block + half of V + first Q block, then the
        # attention stream starts; every other projection and the output
        # projection ride the filler queue inside PE stall points.
        kb = _nblocks(sk)
        qb = _nblocks(S)
        vhalf = (nskc + 1) // 2
        proj_kq(wkt, ktile, 0, xkt, *kb[0])
        for si in range(vhalf):
            proj_v(si)
        proj_kq(wqt, qtile, 0, xt, *qb[0])

        filler = []
        kun = [unit_kq(wkt, ktile, 0, o, n, xkt) for (o, n) in kb[1:]]
        vun = [(lambda si: lambda: proj_v(si))(si) for si in range(vhalf, nskc)]
        while kun or vun:                      # K0b1, Vc, K0b2, Vc, Vc, ...
            if kun:
                filler.append(kun.pop(0))
            if vun:
                filler.append(vun.pop(0))

        attn_head(0, 0, 0, filler, pops=2)
        filler += [unit_kq(wqt, qtile, 0, o, n, xt) for (o, n) in qb[1:]]
        filler += [unit_kq(wkt, ktile, 1, o, n, xkt) for (o, n) in kb]
        filler += [unit_kq(wqt, qtile, 1, o, n, xt) for (o, n) in qb]
        attn_head(0, 0, 1, filler)
        for j in range(1, NS):
            attn_head(0, j, 0, filler)
            attn_head(0, j, 1, filler)

        for j in range(NS):
            attn_head(1, j, 0, filler)
            if j >= 1:                          # normalize(1, j-1) was emitted
                filler += [(lambda si: lambda: outproj_si(si))(si)
                           for si in range(4 * (j - 1), 4 * (j - 1) + 4)]
            attn_head(1, j, 1, filler)

        pending[0]()                            # last head's tail + normalize
        pending[0] = None
        while filler:
            filler.pop(0)()
        for si in range(4 * (NS - 1), 4 * NS):
            outproj_si(si)

    nc.compile()
    return nc


def kernel(x, mask, Wq, bq, Wk, bk, Wv, bv, Wo, bo):
    x = np.asarray(x, np.float32)
    mask = np.asarray(mask)
    Wq, bq = np.asarray(Wq, np.float32), np.asarray(bq, np.float32)
    Wk, bk = np.asarray(Wk, np.float32), np.asarray(bk, np.float32)
    Wv, bv = np.asarray(Wv, np.float32), np.asarray(bv, np.float32)
    Wo, bo = np.asarray(Wo, np.float32), np.asarray(bo, np.float32)

    aug = any(np.any(bias != 0) for bias in (bq, bk, bv))
    kq = 5 if aug else 4

    counts = mask.sum(axis=1)
    sk = max(P, int(-(-max(int(c) for c in counts) // P) * P))
    sk = min(sk, S)
    nskc = sk // P

    in_maps = []
    for c in range(8):
        b, half = c // 2, c % 2
        hs = slice(half * HPC * HD, (half + 1) * HPC * HD)   # 256 head columns

        idx = np.nonzero(mask[b])[0]
        su = len(idx)

        xT = np.zeros((kq * P, S), np.float32)
        xT[:D] = x[b].T
        xKT = np.zeros((5 * P, sk), np.float32)
        xKT[:D, :su] = x[b].T[:, idx]
        xKT[D, :su] = 1.0                      # real-key indicator row
        if aug:
            xT[D] = 1.0

        wq_a = np.zeros((kq * P, HPC * HD), np.float32)
        wq_a[:D] = Wq[:, hs]
        wk_a = np.zeros((kq * P, HPC * HD), np.float32)
        wk_a[:D] = Wk[:, hs]

        wv_a = np.zeros((5 * P, HPC * VB), np.float32)
        for l in range(HPC):
            hg = half * HPC + l
            wv_a[:D, l * VB:l * VB + HD] = Wv[:, hg * HD:(hg + 1) * HD]
            wv_a[D, l * VB + HD] = 1.0         # indicator -> ones column

        if aug:
            wq_a[D] = bq[hs]
            wk_a[D] = bk[hs]
            for l in range(HPC):
                hg = half * HPC + l
                wv_a[D, l * VB:l * VB + HD] = bv[hg * HD:(hg + 1) * HD]

        wo_a = np.stack(
            [Wo[(half * HPC + 2 * m) * HD:(half * HPC + 2 * m + 2) * HD, :]
             for m in range(2)]
        ).astype(np.float32)

        in_maps.append({
            "xT": xT.astype(NPBF16), "xKT": xKT.astype(NPBF16),
            "wq": wq_a.astype(NPBF16), "wk": wk_a.astype(NPBF16),
            "wv": wv_a.astype(NPBF16), "wo": wo_a.astype(NPBF16),
        })

    nc = _build(aug, nskc)
    import os
    trace = bool(int(os.environ.get("MHA_TRACE", "0")))
    res = bass_utils.run_bass_kernel_spmd(nc, in_maps, core_ids=list(range(8)),
                                          trace=trace)
    global last_result
    last_result = res

    outf = np.empty((B, S, D), np.float32)
    for b in range(B):
        outf[b] = res.results[2 * b]["out"] + res.results[2 * b + 1]["out"] + bo[None, :]
    return outf


# revision 22
# speedup vs baseline: 1.3199x; 1.3199x over previous
"""Multi-head attention (B=4, S=2048, D=512, H=8) on 8 Trainium2 NeuronCores.

Sharding: core c handles batch b = c//2 and heads [4*(c%2) .. 4*(c%2)+3]
(data parallel on B, tensor parallel on H). Each core computes Q/K/V
projections for its 4 heads, per-head attention, and a partial output
projection (its 256 rows of Wo). The host sums the two partial outputs per
batch and adds bo.

Perf design (v4, all-bf16):
 - All matmul operands bf16 (fp32 lowers to two half-speed PE passes = 4
   cyc/col; bf16 streams at 1). PSUM accumulation stays fp32. fp8 was
   tried and rejected: quantization noise does not average out for queries
   with concentrated softmax (rel err 5e-2 > the 2e-2 gate).
 - Key compaction on host: masked keys contribute nothing (their V' rows
   incl. the ones-column are zero), so only unmasked keys ship for the K/V
   side, padded to a multiple of 128. ~2x less scores/exp/AV work.
 - The compacted x^T carries an indicator row (1=real key) and wv' a
   matching entry, so V's mask/ones column falls out of the projection -
   no device-side mask work at all.
 - exp runs on the Scalar(ACT) engine over [128,3,512] PSUM spans (three
   key chunks per instruction) to amortize the ~190ns/instr access
   latency. Scores are computed transposed (keys on partitions); softmax
   max-subtraction is skipped (logits ~N(0,1), fp32 psum cannot overflow).
 - The in-order PE stalls whenever it waits on ACT; a filler queue emits
   projection / output-projection chains at those points, and each head's
   last AV group + copy-out ride inside the NEXT head's first group
   (cross-head software pipeline), so the PE never idles. An idle PE also
   drops out of its 2.4GHz p-state, which doubles matmul time.
 - All DMAs are issued from the otherwise-idle GpSimd queue (~25ns issue
   vs ~565ns on the Sync engine, whose serial issue dominated the ramp).
 - Normalization is DRAM-free: the denominator row (V'-ones column of the
   AV matmul) is reciprocal'd on DVE into a single partition, broadcast
   across 64 hd partitions by a K=1 outer-product matmul into PSUM, and
   multiplied into the O tiles; the output projection rides the filler
   queue right behind it.
"""

import numpy as np
import ml_dtypes
from contextlib import ExitStack

import concourse.bass as bass
from concourse.bacc import Bacc
import concourse.mybir as mybir
import concourse.tile as tile
from concourse import bass_utils

F32 = mybir.dt.float32
BF16 = mybir.dt.bfloat16
NPBF16 = ml_dtypes.bfloat16

B, S, D, H, HD = 4, 2048, 512, 8, 64
P = 128
HPC = 4            # heads per core
NS = S // 512      # 4 query blocks of 512
VB = 65            # V' head block: 64 hd cols + the ones/indicator column


def _nblocks(total, step=512):
    return [(o, min(step, total - o)) for o in range(0, total, step)]


def _build(aug: bool, nskc: int) -> bass.Bass:
    kq = 5 if aug else 4           # x^T chunks for the Q/K projections
    sk = nskc * P                  # compacted+padded key count
    # exp groups of <=3 chunks (3 psum banks per group, 2 groups in flight)
    groups = []
    c = 0
    while c < nskc:
        n = min(3, nskc - c)
        groups.append((c, n))
        c += n
    nc = Bacc(trn_type="TRN2")

    xT = nc.dram_tensor("xT", [kq * P, S], BF16, kind="ExternalInput")
    xKT = nc.dram_tensor("xKT", [5 * P, sk], BF16, kind="ExternalInput")
    wq = nc.dram_tensor("wq", [kq * P, HPC * HD], BF16, kind="ExternalInput")
    wk = nc.dram_tensor("wk", [kq * P, HPC * HD], BF16, kind="ExternalInput")
    wv = nc.dram_tensor("wv", [5 * P, HPC * VB], BF16, kind="ExternalInput")
    wo = nc.dram_tensor("wo", [2, P, D], BF16, kind="ExternalInput")
    out = nc.dram_tensor("out", [S, D], F32, kind="ExternalOutput")

    with tile.TileContext(nc) as tc, ExitStack() as ctx:
        sb = ctx.enter_context(tc.tile_pool(name="sb", bufs=1))
        apool = ctx.enter_context(tc.tile_pool(name="sc_ps", bufs=2, space="PSUM"))
        avpool = ctx.enter_context(tc.tile_pool(name="av_ps", bufs=2, space="PSUM"))
        dma = nc.gpsimd.dma_start
        _qs = [nc.gpsimd]
        _qi = [0]

        def ldma(dst, srcap):
            # round-robin input loads over two issue queues: a single
            # sequencer spends ~600ns just issuing each DMA
            _qs[_qi[0] % 1].dma_start(dst, srcap)
            _qi[0] += 1

        _psn = [0]

        def psum512():
            # [128,512] fp32 psum scratch carved from the big "sc" tag
            _psn[0] += 1
            t = apool.tile([P, 3, 512], F32, tag="sc", name=f"ps{_psn[0]}")
            return t[:, 0, :]

        # ---------- input DMAs (split so compute starts early) ----------
        wkt = sb.tile([P, kq, HPC * HD], BF16)
        for t in range(kq):
            ldma(wkt[:, t, :], wk[t * P:(t + 1) * P, :])
        xkt = [sb.tile([P, sk], BF16, tag=f"xk{t}", name=f"xk{t}") for t in range(5)]
        for (o, n) in _nblocks(sk):
            for t in range(5):
                ldma(xkt[t][:, o:o + n], xKT[t * P:(t + 1) * P, o:o + n])
        wvt = sb.tile([P, 5, HPC * VB], BF16)
        for t in range(5):
            ldma(wvt[:, t, :], wv[t * P:(t + 1) * P, :])
        wqt = sb.tile([P, kq, HPC * HD], BF16)
        for t in range(kq):
            ldma(wqt[:, t, :], wq[t * P:(t + 1) * P, :])
        xt = [sb.tile([P, S], BF16, tag=f"xt{t}", name=f"xt{t}") for t in range(kq)]
        for (o, n) in _nblocks(S):
            for t in range(kq):
                ldma(xt[t][:, o:o + n], xT[t * P:(t + 1) * P, o:o + n])
        wot = [sb.tile([P, D], BF16, tag=f"wo{m}", name=f"wo{m}") for m in range(2)]
        for m in range(2):
            ldma(wot[m][:], wo[m])
        ones_t = sb.tile([P, HD], BF16, tag="ones")
        nc.vector.memset(ones_t[:], 1.0)

        # ---------- projections ----------
        ktile = [sb.tile([P, sk], BF16, tag=f"kT{m}", name=f"kT{m}") for m in range(2)]
        qtile = [sb.tile([P, S], BF16, tag=f"qT{m}", name=f"qT{m}") for m in range(2)]
        vt = sb.tile([P, HPC, nskc, VB], BF16)

        def proj_kq(wt, dst, m, src, o, n):
            ps = psum512()
            for t in range(kq):
                nc.tensor.matmul(
                    ps[:, :n], wt[:, t, m * P:(m + 1) * P], src[t][:, o:o + n],
                    start=(t == 0), stop=(t == kq - 1),
                )
            nc.vector.tensor_copy(dst[m][:, o:o + n], ps[:, :n])

        def proj_v(si):
            ps = psum512()
            for t in range(5):
                nc.tensor.matmul(
                    ps[:, :HPC * VB], xkt[t][:, si * P:(si + 1) * P], wvt[:, t, :],
                    start=(t == 0), stop=(t == 4),
                )
            nc.vector.tensor_copy(
                vt[:, :, si, :],
                ps[:, :HPC * VB].rearrange("p (l e) -> p l e", e=VB))

        # ---------- attention ----------
        opair = [sb.tile([P, S], BF16, tag=f"op{m}", name=f"op{m}") for m in range(2)]
        pending = [None]               # previous head's tail (last AVs, copies)
        rrs = {}                       # head parity -> reciprocal tile

        def normalize(m, j):
            # 1/den broadcast to [128, 512] by two K=1 outer products (even
            # head -> partitions 0..63, odd -> 64..127), then one multiply.
            rb = psum512()
            nc.tensor.matmul(rb[0:HD, :], ones_t[HD:HD + 1, :],
                             rrs[0][HD:HD + 1, :], start=True, stop=True)
            nc.tensor.matmul(rb[HD:P, :], ones_t[HD:HD + 1, :],
                             rrs[1][HD:HD + 1, :], start=True, stop=True,
                             skip_group_check=True)
            sl = slice(j * 512, (j + 1) * 512)
            nc.vector.tensor_tensor(opair[m][:, sl], opair[m][:, sl], rb[:],
                                    mybir.AluOpType.mult)

        def attn_head(m, j, h, filler=None, pops=1):
            # Filler units + the previous head's tail are emitted right
            # after the first exp: the in-order PE would otherwise idle
            # there waiting on the ACT engine.
            l = 2 * m + h
            base = h * HD
            qs = qtile[m][base:base + HD, j * 512:(j + 1) * 512]
            av = avpool.tile([VB, 512], F32, tag="av")
            p = sb.tile([P, nskc, 512], BF16, tag="p", bufs=2)
            state = [0]

            def emit_av(upto):
                while state[0] < upto:
                    ck = state[0]
                    nc.tensor.matmul(
                        av[:], vt[:, l, ck, :], p[:, ck, :],
                        start=(ck == 0), stop=(ck == nskc - 1))
                    state[0] += 1

            for gi, (c0, cn) in enumerate(groups):
                sc = apool.tile([P, 3, 512], F32, tag="sc")
                for ci in range(cn):
                    ck = c0 + ci
                    nc.tensor.matmul(
                        sc[:, ci, :],
                        ktile[m][base:base + HD, ck * P:(ck + 1) * P],
                        qs, start=True, stop=True,
                    )
                nc.scalar.activation(p[:, c0:c0 + cn, :], sc[:, :cn, :],
                                     mybir.ActivationFunctionType.Exp,
                                     scale=0.125)
                if filler:
                    for _ in range(pops):
                        if filler:
                            filler.pop(0)()
                if gi == 0 and pending[0] is not None:
                    pending[0]()
                    pending[0] = None
                emit_av(c0)

            def tail():
                emit_av(nskc)
                # O rows into the pair tile (odd head partition-shifted
                # 0..63 -> 64..127 by an SBUF->SBUF DMA) + 1/denominator
                if h == 0:
                    nc.vector.tensor_copy(
                        opair[m][0:HD, j * 512:(j + 1) * 512], av[0:HD, :])
                else:
                    osh = sb.tile([HD, 512], BF16, tag="osh", bufs=2)
                    nc.vector.tensor_copy(osh[:], av[0:HD, :])
                    dma(opair[m][HD:P, j * 512:(j + 1) * 512], osh[:])
                rf = sb.tile([VB, 512], F32, tag=f"rf{h}", bufs=2,
                             name=f"rf{h}_{m}_{j}")
                # base partition must be 0 for the custom DVE op; rows
                # 0..63 produce unused garbage reciprocals of O values
                nc.vector.reciprocal_approx_fast(rf[0:VB, :], av[0:VB, :])
                rr = sb.tile([VB, 512], BF16, tag=f"rr{h}", bufs=2,
                             name=f"rr{h}_{m}_{j}")
                nc.vector.tensor_copy(rr[HD:VB, :], rf[HD:VB, :])
                rrs[h] = rr
                if h == 1:
                    normalize(m, j)

            pending[0] = tail

        def outproj_si(si):
            ps = psum512()
            for mm in range(2):
                nc.tensor.matmul(
                    ps[:], opair[mm][:, si * P:(si + 1) * P], wot[mm][:],
                    start=(mm == 0), stop=(mm == 1),
                )
            osb = sb.tile([P, D], F32, tag="osb", bufs=3)
            nc.vector.tensor_copy(osb[:], ps[:])
            dma(out[si * P:(si + 1) * P, :], osb[:])

        def unit_kq(wt, dst, m, o, n, src):
            return lambda: proj_kq(wt, dst, m, src, o, n)

        # ---------- emission schedule ----------
        # Ramp: first K block + half of V + first Q block, then the
        # attention stream starts; every other projection and the output
        # projection ride the filler queue inside PE stall points.
        kb = _nblocks(sk)
        qb = _nblocks(S)
        vhalf = (nskc + 1) // 2
        proj_kq(wkt, ktile, 0, xkt, *kb[0])
        for si in range(vhalf):
            proj_v(si)
        proj_kq(wqt, qtile, 0, xt, *qb[0])

        filler = []
        kun = [unit_kq(wkt, ktile, 0, o, n, xkt) for (o, n) in kb[1:]]
        vun = [(lambda si: lambda: proj_v(si))(si) for si in range(vhalf, nskc)]
        while kun or vun:                      # K0b1, Vc, K0b2, Vc, Vc, ...
            if kun:
                filler.append(kun.pop(0))
            if vun:
                filler.append(vun.pop(0))

        attn_head(0, 0, 0, filler, pops=2)
        filler += [unit_kq(wqt, qtile, 0, o, n, xt) for (o, n) in qb[1:]]
        filler += [unit_kq(wkt, ktile, 1, o, n, xkt) for (o, n) in kb]
        filler += [unit_kq(wqt, qtile, 1, o, n, xt) for (o, n) in qb]
        attn_head(0, 0, 1, filler)
        for j in range(1, NS):
            attn_head(0, j, 0, filler)
            attn_head(0, j, 1, filler)

        for j in range(NS):
            attn_head(1, j, 0, filler)
            if j >= 1:                          # normalize(1, j-1) was emitted
                filler += [(lambda si: lambda: outproj_si(si))(si)
                           for si in range(4 * (j - 1), 4 * (j - 1) + 4)]
            attn_head(1, j, 1, filler)

        pending[0]()                            # last head's tail + normalize
        pending[0] = None
        while filler:
            filler.pop(0)()
        for si in range(4 * (NS - 1), 4 * NS):
            outproj_si(si)

    nc.compile()
    return nc


def kernel(x, mask, Wq, bq, Wk, bk, Wv, bv, Wo, bo):
    x = np.asarray(x, np.float32)
    mask = np.asarray(mask)
    Wq, bq = np.asarray(Wq, np.float32), np.asarray(bq, np.float32)
    Wk, bk = np.asarray(Wk, np.float32), np.asarray(bk, np.float32)
    Wv, bv = np.asarray(Wv, np.float32), np.asarray(bv, np.float32)
    Wo, bo = np.asarray(Wo, np.float32), np.asarray(bo, np.float32)

    aug = any(np.any(bias != 0) for bias in (bq, bk, bv))
    kq = 5 if aug else 4

    counts = mask.sum(axis=1)
    sk = max(P, int(-(-max(int(c) for c in counts) // P) * P))
    sk = min(sk, S)
    nskc = sk // P

    in_maps = []
    for c in range(8):
        b, half = c // 2, c % 2
        hs = slice(half * HPC * HD, (half + 1) * HPC * HD)   # 256 head columns

        idx = np.nonzero(mask[b])[0]
        su = len(idx)

        xT = np.zeros((kq * P, S), np.float32)
        xT[:D] = x[b].T
        xKT = np.zeros((5 * P, sk), np.float32)
        xKT[:D, :su] = x[b].T[:, idx]
        xKT[D, :su] = 1.0                      # real-key indicator row
        if aug:
            xT[D] = 1.0

        wq_a = np.zeros((kq * P, HPC * HD), np.float32)
        wq_a[:D] = Wq[:, hs]
        wk_a = np.zeros((kq * P, HPC * HD), np.float32)
        wk_a[:D] = Wk[:, hs]

        wv_a = np.zeros((5 * P, HPC * VB), np.float32)
        for l in range(HPC):
            hg = half * HPC + l
            wv_a[:D, l * VB:l * VB + HD] = Wv[:, hg * HD:(hg + 1) * HD]
            wv_a[D, l * VB + HD] = 1.0         # indicator -> ones column

        if aug:
            wq_a[D] = bq[hs]
            wk_a[D] = bk[hs]
            for l in range(HPC):
                hg = half * HPC + l
                wv_a[D, l * VB:l * VB + HD] = bv[hg * HD:(hg + 1) * HD]

        wo_a = np.stack(
            [Wo[(half * HPC + 2 * m) * HD:(half * HPC + 2 * m + 2) * HD, :]
             for m in range(2)]
        ).astype(np.float32)

        in_maps.append({
            "xT": xT.astype(NPBF16), "xKT": xKT.astype(NPBF16),
            "wq": wq_a.astype(NPBF16), "wk": wk_a.astype(NPBF16),
            "wv": wv_a.astype(NPBF16), "wo": wo_a.astype(NPBF16),
        })

    nc = _build(aug, nskc)
    import os
    trace = bool(int(os.environ.get("MHA_TRACE", "0")))
    res = bass_utils.run_bass_kernel_spmd(nc, in_maps, core_ids=list(range(8)),
                                          trace=trace)
    global last_result
    last_result = res

    outf = np.empty((B, S, D), np.float32)
    for b in range(B):
        outf[b] = res.results[2 * b]["out"] + res.results[2 * b + 1]["out"] + bo[None, :]
    return outf
